# revision 31
# baseline (speedup 1.0000x reference)
"""GAT layer (nn_GATLayer) on 8 Trainium2 NeuronCores.

Math (per batch b, with h = x@W, s1 = h@a1, s2 = h@a2):
    e[i,j] = lrelu_0.2(s1_i + s2_j), masked by adj[i,j], softmax over j
    out    = attn @ h

Key identity: softmax over j is invariant to any per-i scale, and
exp(lrelu(y)) = max(exp(y), exp(0.2 y)). Dividing column i by e^{0.2 s1_i}:
    P'[j,i] = max(e^{0.8 s1_i} * e^{s2_j},  e^{0.2 s2_j}) * adj[i,j]
-- the i-dependence is a rank-1 product: no N^2 transcendentals at all.

Device formulation (per core = one batch element), [p=j, f=i] layout:
    q   = (E1b mult E2_j) max F2_j   (tensor_scalar: 4x DVE perf mode;
                                      a few tiles run on gpsimd instead)
    P'' = mask * q                   (tensor_tensor over 2-tile pairs,
                                      2x DVE perf mode, {0,1} bf16 mask)
    numT[d,i] = sum_j hcat[j,d] P''[j,i],  hcat = [h | 1]  (PE, bf16)
Host computes h/s1/s2/exp vectors (O(N D^2 + N) work) and the final
divide+transpose out[i,d] = numT[d,i]/numT[64,i].

Sharding: data-parallel over batch B=8 across the 8 cores; mask (shared)
replicated. All N^2 element work uses standard DVE/Pool ops; the DVE ones
hit the 2x/4x perf modes (bf16, packed, SBUF) -- custom DVE ops can't.
"""

import os
import sys

sys.path.insert(0, "/opt/trn_rl_repo")

import numpy as np
import ml_dtypes

B, N, DIN, DOUT = 8, 2048, 64, 64
NCORES = 8
PJ = 128              # j-tile partition size
NJT = N // PJ         # 16 j-tiles
NPAIR = NJT // 2      # mask/score processed in 2-tile pairs
FCH = 512             # psum bank chunk (fp32)
NCH = N // FCH        # 4 chunks of the free dim per tile
HC = DOUT + 2         # hcat stride: 64 h cols + 1 ones col + 1 pad

_COMPILED = None
LAST_RESULT = None    # BassKernelResults from the last run (for test.py)


def _build_nc():
    """Build the Bass module (shared SPMD program for all 8 cores)."""
    from contextlib import ExitStack

    import concourse.tile as tile
    from concourse import bacc, mybir

    f32 = mybir.dt.float32
    bf16 = mybir.dt.bfloat16
    ALU = mybir.AluOpType

    nc = bacc.Bacc("TRN2", target_bir_lowering=False, debug=False, num_devices=NCORES)

    # maskp[p, t*N + i] = adjT[t*128+p, i] -- pre-tiled on host so each
    # 2-tile pair is one contiguous 2D transfer (single writer per buffer;
    # multi-writer buffers raced their DMA completion semaphores)
    maskp = nc.dram_tensor("maskp", [PJ, NJT * N], bf16, kind="ExternalInput").ap()
    hcat = nc.dram_tensor("hcat", [PJ, NJT * HC], bf16, kind="ExternalInput").ap()
    e1b = nc.dram_tensor("e1b", [PJ, N], bf16, kind="ExternalInput").ap()
    e2f2 = nc.dram_tensor("e2f2", [PJ, 2 * NJT], f32, kind="ExternalInput").ap()
    out = nc.dram_tensor("out", [DOUT + 1, N], f32, kind="ExternalOutput").ap()

    with ExitStack() as ctx:
        tc = ctx.enter_context(tile.TileContext(nc))

        const = ctx.enter_context(tc.tile_pool(name="const", bufs=1))
        big = ctx.enter_context(tc.tile_pool(name="big", bufs=1))

        # ---- single DMA queue: scalars, then pair0 masks, hcat, rest ----
        e1b_sb = big.tile([PJ, N], bf16, tag="e1b")
        nc.sync.dma_start(e1b_sb[:], e1b)
        e2f2_sb = const.tile([PJ, 2 * NJT], f32, tag="e2f2")
        nc.sync.dma_start(e2f2_sb[:], e2f2)

        mpool = ctx.enter_context(tc.tile_pool(name="mask", bufs=NPAIR))
        mask_sb = []
        hcat_sb = None
        for g in range(NPAIR):
            mb_g = mpool.tile([PJ, 2 * N], bf16, tag="mb")
            if g == 0:
                # two half transfers so TT0 ungates as early as possible
                nc.sync.dma_start(mb_g[:, :N], maskp[:, :N])
                nc.sync.dma_start(mb_g[:, N:], maskp[:, N : 2 * N])
            else:
                nc.sync.dma_start(mb_g[:], maskp[:, 2 * g * N : (2 * g + 2) * N])
            mask_sb.append(mb_g)
            if g == 0:
                # hcat is only needed by the first matmul, well after TT0
                hcat_sb = const.tile([PJ, NJT * HC], bf16, tag="hcat")
                nc.sync.dma_start(hcat_sb[:], hcat)

        num_pool = ctx.enter_context(
            tc.tile_pool(name="num_psum", bufs=1, space="PSUM")
        )
        numT_ps = num_pool.tile([DOUT + 1, N], f32, tag="numt")

        qpool = ctx.enter_context(tc.tile_pool(name="q", bufs=3))
        ppool = ctx.enter_context(tc.tile_pool(name="probs", bufs=3))

        # matmuls MUST be emitted inline per pair: pool WAR tracking follows
        # program order, so a deferred reader of a recycled buffer is
        # invisible to the pool and gets overwritten (a real race on HW)
        for g in range(NPAIR):
            q_sb = qpool.tile([PJ, 2 * N], bf16, tag="q")
            for k in range(2):
                t = 2 * g + k
                nc.vector.tensor_scalar(
                    q_sb[:, k * N : (k + 1) * N],
                    e1b_sb[:],
                    e2f2_sb[:, t : t + 1],
                    e2f2_sb[:, NJT + t : NJT + t + 1],
                    op0=ALU.mult,
                    op1=ALU.max,
                )
            p_sb = ppool.tile([PJ, 2 * N], bf16, tag="p")
            if g == 0:
                # two single-tile multiplies: TT0 ungates on the first
                # half-DMA alone, starting PE ~1.5us earlier
                halves = [slice(0, N), slice(N, 2 * N)]
            else:
                halves = [slice(0, 2 * N)]
            for h in halves:
                nc.vector.tensor_tensor(
                    p_sb[:, h], mask_sb[g][:, h], q_sb[:, h], op=ALU.mult
                )

            for k in range(2):
                t = 2 * g + k
                lhsT = hcat_sb[:, t * HC : t * HC + DOUT + 1]
                for c in range(NCH):
                    sl = slice(c * FCH, (c + 1) * FCH)
                    # every 512-col PSUM chunk needs its own start/stop:
                    # start resets that bank region's accumulator
                    nc.tensor.matmul(
                        numT_ps[:, sl],
                        lhsT,
                        p_sb[:, k * N + c * FCH : k * N + (c + 1) * FCH],
                        start=(t == 0),
                        stop=(t == NJT - 1),
                    )

        # ---- drain numT (ACT and DVE in parallel, one writer per chunk)
        # and store; divide+transpose happen on host ----
        numt_sb = big.tile([DOUT + 1, N], f32, tag="numt_sb")
        for c in range(NCH):
            sl = slice(c * FCH, (c + 1) * FCH)
            if c % 2 == 0:
                nc.scalar.copy(numt_sb[:, sl], numT_ps[:, sl])
            else:
                nc.vector.tensor_copy(numt_sb[:, sl], numT_ps[:, sl])
            nc.sync.dma_start(out[:, sl], numt_sb[:, sl])

    nc.compile()
    return nc


def _prep_inputs(x, adj, W, a):
    bf = ml_dtypes.bfloat16
    x = np.asarray(x, dtype=np.float32)
    W = np.ascontiguousarray(np.asarray(W, dtype=np.float32))
    a = np.asarray(a, dtype=np.float32)

    h = x @ W                                   # [B,N,DOUT]
    s1 = h @ a[:DOUT]                           # [B,N]
    s2 = h @ a[DOUT:]                           # [B,N]

    adjT = (np.asarray(adj).T > 0).astype(bf)   # [j,i] layout, {0,1}
    # pre-tiled [128, 16*2048]: maskp[p, t*N+i] = adjT[t*128+p, i]
    maskp = np.ascontiguousarray(
        adjT.reshape(NJT, PJ, N).transpose(1, 0, 2).reshape(PJ, NJT * N)
    )

    in_maps = []
    for b in range(NCORES):
        hcat = np.zeros((N, HC), dtype=bf)
        hcat[:, :DOUT] = h[b].astype(bf)
        hcat[:, DOUT] = bf(1.0)
        # pre-tiled [128, 16*66]: partition p, tile t = row t*128+p
        hcat_t = np.ascontiguousarray(
            hcat.reshape(NJT, PJ, HC).transpose(1, 0, 2).reshape(PJ, NJT * HC)
        )
        e1p = np.exp(0.8 * s1[b]).astype(bf)
        e1b = np.ascontiguousarray(np.broadcast_to(e1p[None, :], (PJ, N)))
        e2 = np.exp(s2[b]).astype(np.float32).reshape(NJT, PJ).T
        f2 = np.exp(0.2 * s2[b]).astype(np.float32).reshape(NJT, PJ).T
        e2f2 = np.ascontiguousarray(np.concatenate([e2, f2], axis=1))
        in_maps.append(
            {"maskp": maskp, "hcat": hcat_t, "e1b": e1b, "e2f2": e2f2}
        )
    return in_maps


def kernel(x, adj, W, a):
    global _COMPILED, LAST_RESULT
    from concourse import bass_utils

    x = np.asarray(x)
    adj = np.asarray(adj)
    assert x.shape == (B, N, DIN) and adj.shape == (N, N)

    if _COMPILED is None:
        _COMPILED = _build_nc()
    nc = _COMPILED

    in_maps = _prep_inputs(x, adj, W, a)
    res = bass_utils.run_bass_kernel_spmd(
        nc,
        in_maps,
        core_ids=list(range(NCORES)),
        trace=bool(int(os.environ.get("GAT_TRACE", "0"))),
    )
    LAST_RESULT = res
    out = np.empty((B, N, DOUT), dtype=np.float32)
    for b in range(NCORES):
        numt = res.results[b]["out"]            # [DOUT+1, N] f32
        out[b] = (numt[:DOUT] / numt[DOUT : DOUT + 1]).T
    return out


# revision 33
# speedup vs baseline: 1.0157x; 1.0157x over previous
"""GAT layer (nn_GATLayer) on 8 Trainium2 NeuronCores.

Math (per batch b, with h = x@W, s1 = h@a1, s2 = h@a2):
    e[i,j] = lrelu_0.2(s1_i + s2_j), masked by adj[i,j], softmax over j
    out    = attn @ h

Key identity: softmax over j is invariant to any per-i scale, and
exp(lrelu(y)) = max(exp(y), exp(0.2 y)). Dividing column i by e^{0.2 s1_i}:
    P'[j,i] = max(e^{0.8 s1_i} * e^{s2_j},  e^{0.2 s2_j}) * adj[i,j]
-- the i-dependence is a rank-1 product: no N^2 transcendentals at all.

Device formulation (per core = one batch element), [p=j, f=i] layout:
    q   = (E1b mult E2_j) max F2_j   (tensor_scalar: 4x DVE perf mode)
    P'' = mask * q                   (tensor_tensor over 2-tile pairs,
                                      2x DVE perf mode, {0,1} bf16 mask)
    numT[d,i] = sum_j hcat[j,d] P''[j,i],  hcat = [h | 1]  (PE, bf16)
Host computes h/s1/s2/exp vectors (O(N D^2 + N) work) and the final
divide+transpose out[i,d] = numT[d,i]/numT[64,i].

Sharding: data-parallel over batch B=8 across the 8 cores; mask (shared)
replicated. All N^2 element work runs as standard DVE ops that hit the
2x/4x perf modes (bf16, packed, SBUF) -- custom DVE ops can't, ACT exp
over N^2 is gone entirely, and gpsimd is avoided (its elementwise ops
time-slice against the DVE datapath, stalling both).
"""

import os
import sys

sys.path.insert(0, "/opt/trn_rl_repo")

import numpy as np
import ml_dtypes

B, N, DIN, DOUT = 8, 2048, 64, 64
NCORES = 8
PJ = 128              # j-tile partition size
NJT = N // PJ         # 16 j-tiles
NPAIR = NJT // 2      # mask/score processed in 2-tile pairs
FCH = 512             # psum bank chunk (fp32)
NCH = N // FCH        # 4 chunks of the free dim per tile
HC = DOUT + 2         # hcat stride: 64 h cols + 1 ones col + 1 pad

_COMPILED = None
LAST_RESULT = None    # BassKernelResults from the last run (for test.py)


def _build_nc():
    """Build the Bass module (shared SPMD program for all 8 cores)."""
    from contextlib import ExitStack

    import concourse.tile as tile
    from concourse import bacc, mybir

    f32 = mybir.dt.float32
    bf16 = mybir.dt.bfloat16
    ALU = mybir.AluOpType

    nc = bacc.Bacc("TRN2", target_bir_lowering=False, debug=False, num_devices=NCORES)

    # maskp[p, t*N + i] = adjT[t*128+p, i] -- pre-tiled on host so each
    # 2-tile pair is one contiguous 2D transfer (single writer per buffer;
    # multi-writer buffers raced their DMA completion semaphores)
    maskp = nc.dram_tensor("maskp", [PJ, NJT * N], bf16, kind="ExternalInput").ap()
    hcat = nc.dram_tensor("hcat", [PJ, NJT * HC], bf16, kind="ExternalInput").ap()
    e1b = nc.dram_tensor("e1b", [PJ, N], bf16, kind="ExternalInput").ap()
    e2f2 = nc.dram_tensor("e2f2", [PJ, 2 * NJT], f32, kind="ExternalInput").ap()
    out = nc.dram_tensor("out", [DOUT + 1, N], f32, kind="ExternalOutput").ap()

    with ExitStack() as ctx:
        tc = ctx.enter_context(tile.TileContext(nc))

        const = ctx.enter_context(tc.tile_pool(name="const", bufs=1))
        big = ctx.enter_context(tc.tile_pool(name="big", bufs=1))

        # ---- single DMA queue: scalars, then pair0 masks, hcat, rest ----
        e1b_sb = big.tile([PJ, N], bf16, tag="e1b")
        nc.sync.dma_start(e1b_sb[:], e1b)
        e2f2_sb = const.tile([PJ, 2 * NJT], f32, tag="e2f2")
        nc.sync.dma_start(e2f2_sb[:], e2f2)

        mpool = ctx.enter_context(tc.tile_pool(name="mask", bufs=NPAIR))
        mask_sb = []
        hcat_sb = None
        for g in range(NPAIR):
            mb_g = mpool.tile([PJ, 2 * N], bf16, tag="mb")
            if g == 0:
                # two half transfers so TT0 ungates as early as possible
                nc.sync.dma_start(mb_g[:, :N], maskp[:, :N])
                nc.sync.dma_start(mb_g[:, N:], maskp[:, N : 2 * N])
            else:
                nc.sync.dma_start(mb_g[:], maskp[:, 2 * g * N : (2 * g + 2) * N])
            mask_sb.append(mb_g)
            if g == 0:
                # hcat is only needed by the first matmul, well after TT0
                hcat_sb = const.tile([PJ, NJT * HC], bf16, tag="hcat")
                nc.sync.dma_start(hcat_sb[:], hcat)

        num_pool = ctx.enter_context(
            tc.tile_pool(name="num_psum", bufs=1, space="PSUM")
        )
        numT_ps = num_pool.tile([DOUT + 1, N], f32, tag="numt")

        qpool = ctx.enter_context(tc.tile_pool(name="q", bufs=3))
        ppool = ctx.enter_context(tc.tile_pool(name="probs", bufs=3))

        # matmuls MUST be emitted inline per pair: pool WAR tracking follows
        # program order, so a deferred reader of a recycled buffer is
        # invisible to the pool and gets overwritten (a real race on HW)
        for g in range(NPAIR):
            q_sb = qpool.tile([PJ, 2 * N], bf16, tag="q")
            for k in range(2):
                t = 2 * g + k
                nc.vector.tensor_scalar(
                    q_sb[:, k * N : (k + 1) * N],
                    e1b_sb[:],
                    e2f2_sb[:, t : t + 1],
                    e2f2_sb[:, NJT + t : NJT + t + 1],
                    op0=ALU.mult,
                    op1=ALU.max,
                )
            p_sb = ppool.tile([PJ, 2 * N], bf16, tag="p")
            nc.vector.tensor_tensor(p_sb[:], mask_sb[g][:], q_sb[:], op=ALU.mult)

            for k in range(2):
                t = 2 * g + k
                lhsT = hcat_sb[:, t * HC : t * HC + DOUT + 1]
                for c in range(NCH):
                    sl = slice(c * FCH, (c + 1) * FCH)
                    # every 512-col PSUM chunk needs its own start/stop:
                    # start resets that bank region's accumulator
                    nc.tensor.matmul(
                        numT_ps[:, sl],
                        lhsT,
                        p_sb[:, k * N + c * FCH : k * N + (c + 1) * FCH],
                        start=(t == 0),
                        stop=(t == NJT - 1),
                    )

        # ---- drain numT (ACT and DVE in parallel, one writer per chunk)
        # and store; divide+transpose happen on host ----
        numt_sb = big.tile([DOUT + 1, N], f32, tag="numt_sb")
        for c in range(NCH):
            sl = slice(c * FCH, (c + 1) * FCH)
            if c % 2 == 0:
                nc.scalar.copy(numt_sb[:, sl], numT_ps[:, sl])
            else:
                nc.vector.tensor_copy(numt_sb[:, sl], numT_ps[:, sl])
            nc.sync.dma_start(out[:, sl], numt_sb[:, sl])

    nc.compile()
    return nc


def _prep_inputs(x, adj, W, a):
    bf = ml_dtypes.bfloat16
    x = np.asarray(x, dtype=np.float32)
    W = np.ascontiguousarray(np.asarray(W, dtype=np.float32))
    a = np.asarray(a, dtype=np.float32)

    h = x @ W                                   # [B,N,DOUT]
    s1 = h @ a[:DOUT]                           # [B,N]
    s2 = h @ a[DOUT:]                           # [B,N]

    adjT = (np.asarray(adj).T > 0).astype(bf)   # [j,i] layout, {0,1}
    # pre-tiled [128, 16*2048]: maskp[p, t*N+i] = adjT[t*128+p, i]
    maskp = np.ascontiguousarray(
        adjT.reshape(NJT, PJ, N).transpose(1, 0, 2).reshape(PJ, NJT * N)
    )

    in_maps = []
    for b in range(NCORES):
        hcat = np.zeros((N, HC), dtype=bf)
        hcat[:, :DOUT] = h[b].astype(bf)
        hcat[:, DOUT] = bf(1.0)
        # pre-tiled [128, 16*66]: partition p, tile t = row t*128+p
        hcat_t = np.ascontiguousarray(
            hcat.reshape(NJT, PJ, HC).transpose(1, 0, 2).reshape(PJ, NJT * HC)
        )
        e1p = np.exp(0.8 * s1[b]).astype(bf)
        e1b = np.ascontiguousarray(np.broadcast_to(e1p[None, :], (PJ, N)))
        e2 = np.exp(s2[b]).astype(np.float32).reshape(NJT, PJ).T
        f2 = np.exp(0.2 * s2[b]).astype(np.float32).reshape(NJT, PJ).T
        e2f2 = np.ascontiguousarray(np.concatenate([e2, f2], axis=1))
        in_maps.append(
            {"maskp": maskp, "hcat": hcat_t, "e1b": e1b, "e2f2": e2f2}
        )
    return in_maps


def kernel(x, adj, W, a):
    global _COMPILED, LAST_RESULT
    from concourse import bass_utils

    x = np.asarray(x)
    adj = np.asarray(adj)
    assert x.shape == (B, N, DIN) and adj.shape == (N, N)

    if _COMPILED is None:
        _COMPILED = _build_nc()
    nc = _COMPILED

    in_maps = _prep_inputs(x, adj, W, a)
    res = bass_utils.run_bass_kernel_spmd(
        nc,
        in_maps,
        core_ids=list(range(NCORES)),
        trace=bool(int(os.environ.get("GAT_TRACE", "0"))),
    )
    LAST_RESULT = res
    out = np.empty((B, N, DOUT), dtype=np.float32)
    for b in range(NCORES):
        numt = res.results[b]["out"]            # [DOUT+1, N] f32
        out[b] = (numt[:DOUT] / numt[DOUT : DOUT + 1]).T
    return out
